# revision 1
# baseline (speedup 1.0000x reference)
"""Trainium2 Bass kernel for nn_InputRotationWrapper: y = WHT(x) @ W^T + b.

Algebraic fold: WHT (normalized Walsh-Hadamard along feature dim, H symmetric)
commutes into the weight: y = (x H) W^T = x (W H)^T.  The device therefore runs
a pure GEMM  y = x @ Wr^T + b  with Wr = WHT(W) computed once on the host.

Distribution: data-parallel over the 8192 tokens across 8 NeuronCores (1024
tokens each); Wr is replicated.  Each core computes its output slice
transposed (yT[o, t], o on partitions) so every DMA is fully contiguous:
  - x^T shard  [4096 d, 1024 t]  resident in SBUF as fp32r (16.7 MB)
  - Wr packed  [32 ob, 128 d_in, 32 d_chunk, 128 o]  streamed per o-block
  - out yT     [4096 o, 1024 t]  written per o-block
Matmul dtype is float32r (full-rate on the PE at N>=256, ~1.5e-4 rel err).
Bias is fused into the PSUM->SBUF eviction via ScalarE activation.
"""
import sys

for _p in ("/opt/trn_rl_repo", "/root/.axon_site/_ro/trn_rl_repo"):
    if _p not in sys.path:
        sys.path.insert(0, _p)

import numpy as np

D = 4096          # feature dim (= rotation size)
TOKENS = 8192     # 4 * 2048
N_CORES = 8
T_CORE = TOKENS // N_CORES   # 1024 tokens per core
P = 128           # partitions
DC = D // P       # 32 contraction chunks
OB = D // P       # 32 output blocks
T_HALF = 512      # moving free-dim per matmul (fp32 max)

_compiled = None  # (nc, tmpdir) cache


def _matmul_hadU_np(x: np.ndarray) -> np.ndarray:
    """Normalized WHT along the last axis — exact port of the reference
    recursive-butterfly (K == 1 branch), in float64."""
    n = x.shape[-1]
    shape = x.shape
    v = x.reshape(-1, n, 1)
    while v.shape[1] > 1:
        b_, m, c = v.shape
        v = v.reshape(b_, m // 2, 2, c)
        a, b = v[:, :, 0, :], v[:, :, 1, :]
        v = np.concatenate([a + b, a - b], axis=-1)
    return v.reshape(shape) / np.sqrt(n)


def _build_nc():
    import concourse.tile as tile
    from concourse import bacc, mybir

    dt = mybir.dt
    nc = bacc.Bacc(None, target_bir_lowering=False)

    xt_d = nc.dram_tensor("xt", [D, T_CORE], dt.float32, kind="ExternalInput")
    w_d = nc.dram_tensor("w", [OB, P, DC, P], dt.float32, kind="ExternalInput")
    b_d = nc.dram_tensor("bias", [P, OB], dt.float32, kind="ExternalInput")
    y_d = nc.dram_tensor("yt", [D, T_CORE], dt.float32, kind="ExternalOutput")

    with tile.TileContext(nc) as tc:
        with (
            tc.tile_pool(name="xp", bufs=1) as xp,
            tc.tile_pool(name="wp", bufs=2) as wp,
            tc.tile_pool(name="bp", bufs=1) as bp,
            tc.tile_pool(name="op", bufs=3) as op,
            tc.tile_pool(name="pp", bufs=2, space="PSUM") as pp,
        ):
            b_sb = bp.tile([P, OB], dt.float32)
            nc.sync.dma_start(b_sb[:], b_d[:])

            x_tiles = []
            for c in range(DC):
                t = xp.tile([P, T_CORE], dt.float32r, tag=f"x{c}")
                nc.sync.dma_start(
                    t[:], xt_d[c * P:(c + 1) * P, :].bitcast(dt.float32r)
                )
                x_tiles.append(t)

            for ob in range(OB):
                w_sb = wp.tile([P, DC, P], dt.float32r, tag="w")
                nc.sync.dma_start(w_sb[:], w_d[ob].bitcast(dt.float32r))

                ps = pp.tile([P, T_CORE], dt.float32, tag="ps")
                for c in range(DC):
                    lhsT = w_sb[:, c, :]
                    nc.tensor.matmul(
                        ps[:, 0:T_HALF], lhsT, x_tiles[c][:, 0:T_HALF],
                        start=(c == 0), stop=(c == DC - 1),
                    )
                    nc.tensor.matmul(
                        ps[:, T_HALF:T_CORE], lhsT, x_tiles[c][:, T_HALF:T_CORE],
                        start=(c == 0), stop=(c == DC - 1),
                    )

                o_sb = op.tile([P, T_CORE], dt.float32, tag="o")
                nc.scalar.activation(
                    o_sb[:], ps[:],
                    mybir.ActivationFunctionType.Identity,
                    bias=b_sb[:, ob:ob + 1],
                )
                nc.sync.dma_start(y_d[ob * P:(ob + 1) * P, :], o_sb[:])

    nc.compile()
    return nc


def _get_nc():
    global _compiled
    if _compiled is None:
        _compiled = _build_nc()
    return _compiled


def _prep_inputs(x, W, b):
    x = np.asarray(x, dtype=np.float32)
    W = np.asarray(W, dtype=np.float32)
    b = np.asarray(b, dtype=np.float32)

    Wr = _matmul_hadU_np(W.astype(np.float64)).astype(np.float32)  # [o, d]
    # W_pack[ob, p, c, j] = Wr[ob*128 + j, c*128 + p]
    w_pack = np.ascontiguousarray(
        Wr.reshape(OB, P, DC, P).transpose(0, 3, 2, 1)
    )
    b_pack = np.ascontiguousarray(b.reshape(OB, P).T)  # [128, 32]

    xt = np.ascontiguousarray(
        x.reshape(N_CORES, T_CORE, D).transpose(0, 2, 1)
    )  # [8, 4096, 1024]

    in_maps = [
        {"xt": xt[c], "w": w_pack, "bias": b_pack} for c in range(N_CORES)
    ]
    return in_maps


def _assemble(results):
    # yt per core: [4096 o, 1024 t] -> y[t, o]
    parts = [np.ascontiguousarray(r["yt"].T) for r in results]
    y = np.concatenate(parts, axis=0)  # [8192, 4096]
    return y.reshape(4, 2048, D)


def _run(x, W, b, **spmd_kwargs):
    from concourse.bass_utils import run_bass_kernel_spmd

    nc = _get_nc()
    in_maps = _prep_inputs(x, W, b)
    res = run_bass_kernel_spmd(nc, in_maps, list(range(N_CORES)), **spmd_kwargs)
    return _assemble(res.results), res


def kernel(x, W, b):
    out, _ = _run(x, W, b)
    return out


# revision 3
# speedup vs baseline: 1.0436x; 1.0436x over previous
"""Trainium2 Bass kernel for nn_InputRotationWrapper: y = WHT(x) @ W^T + b.

Algebraic fold: WHT (normalized Walsh-Hadamard along feature dim, H symmetric)
commutes into the weight: y = (x H) W^T = x (W H)^T.  The device therefore runs
a pure GEMM  y = x @ Wr^T + b  with Wr = WHT(W) computed once on the host.

Distribution: data-parallel over the 8192 tokens across 8 NeuronCores (1024
tokens each); Wr is replicated.  Each core computes its output slice
transposed (yT[o, t], o on partitions) so every DMA is fully contiguous:
  - x^T shard  [4096 d, 1024 t]  resident in SBUF as fp32r (16.7 MB)
  - Wr packed  [32 ob, 128 d_in, 32 d_chunk, 128 o]  streamed per o-block
  - out yT     [4096 o, 1024 t]  written per o-block
Matmul dtype is float32r (full-rate on the PE at N>=256, ~1.5e-4 rel err).
Bias is fused into the PSUM->SBUF eviction via ScalarE activation.
"""
import sys

for _p in ("/opt/trn_rl_repo", "/root/.axon_site/_ro/trn_rl_repo"):
    if _p not in sys.path:
        sys.path.insert(0, _p)

import numpy as np

D = 4096          # feature dim (= rotation size)
TOKENS = 8192     # 4 * 2048
N_CORES = 8
T_CORE = TOKENS // N_CORES   # 1024 tokens per core
P = 128           # partitions
DC = D // P       # 32 contraction chunks
OB = D // P       # 32 output blocks
T_HALF = 512      # moving free-dim per matmul (fp32 max)

_compiled = None  # (nc, tmpdir) cache


def _matmul_hadU_np(x: np.ndarray) -> np.ndarray:
    """Normalized WHT along the last axis — exact port of the reference
    recursive-butterfly (K == 1 branch), in float64."""
    n = x.shape[-1]
    shape = x.shape
    v = x.reshape(-1, n, 1)
    while v.shape[1] > 1:
        b_, m, c = v.shape
        v = v.reshape(b_, m // 2, 2, c)
        a, b = v[:, :, 0, :], v[:, :, 1, :]
        v = np.concatenate([a + b, a - b], axis=-1)
    return v.reshape(shape) / np.sqrt(n)


def _build_nc():
    import concourse.tile as tile
    from concourse import bacc, mybir

    dt = mybir.dt
    nc = bacc.Bacc(None, target_bir_lowering=False)

    xt_d = nc.dram_tensor("xt", [D, T_CORE], dt.float32, kind="ExternalInput")
    w_d = nc.dram_tensor("w", [OB, P, DC, P], dt.float32, kind="ExternalInput")
    b_d = nc.dram_tensor("bias", [P, OB], dt.float32, kind="ExternalInput")
    y_d = nc.dram_tensor("yt", [D, T_CORE], dt.float32, kind="ExternalOutput")

    G0 = 4  # o-blocks processed c-outer in the startup group (PE saturates
            # while x tiles stream in; needs G0*2 PSUM banks + G0 W slots)

    with tile.TileContext(nc) as tc:
        with (
            tc.tile_pool(name="xp", bufs=1) as xp,
            tc.tile_pool(name="wp", bufs=G0, space="SBUF") as wp,
            tc.tile_pool(name="bp", bufs=1) as bp,
            tc.tile_pool(name="op", bufs=2) as op,
            tc.tile_pool(name="pp", bufs=G0, space="PSUM") as pp,
        ):
            def evict(ob, ps):
                o_sb = op.tile([P, T_CORE], dt.float32, tag="o")
                nc.scalar.activation(
                    o_sb[:], ps[:],
                    mybir.ActivationFunctionType.Identity,
                    bias=b_sb[:, ob:ob + 1],
                )
                nc.sync.dma_start(y_d[ob * P:(ob + 1) * P, :], o_sb[:])

            # W for the startup group first so its matmuls only gate on x
            w0_tiles = []
            for ob in range(G0):
                w_sb = wp.tile([P, DC, P], dt.float32r, tag="w")
                nc.sync.dma_start(w_sb[:], w_d[ob].bitcast(dt.float32r))
                w0_tiles.append(w_sb)

            b_sb = bp.tile([P, OB], dt.float32)
            nc.sync.dma_start(b_sb[:], b_d[:])

            x_tiles = []
            for c in range(DC):
                t = xp.tile([P, T_CORE], dt.float32r, tag=f"x{c}")
                nc.sync.dma_start(
                    t[:], xt_d[c * P:(c + 1) * P, :].bitcast(dt.float32r)
                )
                x_tiles.append(t)

            # startup group: c outer, o-blocks inner -> 2*G0 matmuls become
            # ready the moment x tile c lands
            ps0 = [
                pp.tile([P, T_CORE], dt.float32, tag="ps", name=f"ps0_{i}")
                for i in range(G0)
            ]
            for c in range(DC):
                for ob in range(G0):
                    lhsT = w0_tiles[ob][:, c, :]
                    nc.tensor.matmul(
                        ps0[ob][:, 0:T_HALF], lhsT, x_tiles[c][:, 0:T_HALF],
                        start=(c == 0), stop=(c == DC - 1),
                    )
                    nc.tensor.matmul(
                        ps0[ob][:, T_HALF:T_CORE], lhsT,
                        x_tiles[c][:, T_HALF:T_CORE],
                        start=(c == 0), stop=(c == DC - 1),
                    )
            for ob in range(G0):
                evict(ob, ps0[ob])

            # steady state: one o-block at a time, W double-buffered
            for ob in range(G0, OB):
                w_sb = wp.tile([P, DC, P], dt.float32r, tag="w")
                nc.sync.dma_start(w_sb[:], w_d[ob].bitcast(dt.float32r))

                ps = pp.tile([P, T_CORE], dt.float32, tag="ps")
                for c in range(DC):
                    lhsT = w_sb[:, c, :]
                    nc.tensor.matmul(
                        ps[:, 0:T_HALF], lhsT, x_tiles[c][:, 0:T_HALF],
                        start=(c == 0), stop=(c == DC - 1),
                    )
                    nc.tensor.matmul(
                        ps[:, T_HALF:T_CORE], lhsT, x_tiles[c][:, T_HALF:T_CORE],
                        start=(c == 0), stop=(c == DC - 1),
                    )
                evict(ob, ps)

    nc.compile()
    return nc


def _get_nc():
    global _compiled
    if _compiled is None:
        _compiled = _build_nc()
    return _compiled


def _prep_inputs(x, W, b):
    x = np.asarray(x, dtype=np.float32)
    W = np.asarray(W, dtype=np.float32)
    b = np.asarray(b, dtype=np.float32)

    Wr = _matmul_hadU_np(W.astype(np.float64)).astype(np.float32)  # [o, d]
    # W_pack[ob, p, c, j] = Wr[ob*128 + j, c*128 + p]
    w_pack = np.ascontiguousarray(
        Wr.reshape(OB, P, DC, P).transpose(0, 3, 2, 1)
    )
    b_pack = np.ascontiguousarray(b.reshape(OB, P).T)  # [128, 32]

    xt = np.ascontiguousarray(
        x.reshape(N_CORES, T_CORE, D).transpose(0, 2, 1)
    )  # [8, 4096, 1024]

    in_maps = [
        {"xt": xt[c], "w": w_pack, "bias": b_pack} for c in range(N_CORES)
    ]
    return in_maps


def _assemble(results):
    # yt per core: [4096 o, 1024 t] -> y[t, o]
    parts = [np.ascontiguousarray(r["yt"].T) for r in results]
    y = np.concatenate(parts, axis=0)  # [8192, 4096]
    return y.reshape(4, 2048, D)


def _run(x, W, b, **spmd_kwargs):
    from concourse.bass_utils import run_bass_kernel_spmd

    nc = _get_nc()
    in_maps = _prep_inputs(x, W, b)
    res = run_bass_kernel_spmd(nc, in_maps, list(range(N_CORES)), **spmd_kwargs)
    return _assemble(res.results), res


def kernel(x, W, b):
    out, _ = _run(x, W, b)
    return out


# revision 4
# speedup vs baseline: 1.0702x; 1.0255x over previous
"""Trainium2 Bass kernel for nn_InputRotationWrapper: y = WHT(x) @ W^T + b.

Algebraic fold: WHT (normalized Walsh-Hadamard along feature dim, H symmetric)
commutes into the weight: y = (x H) W^T = x (W H)^T.  The device therefore runs
a pure GEMM  y = x @ Wr^T + b  with Wr = WHT(W) computed once on the host.

Distribution: data-parallel over the 8192 tokens across 8 NeuronCores (1024
tokens each); Wr is replicated.  Each core computes its output slice
transposed (yT[o, t], o on partitions) so every DMA is fully contiguous:
  - x^T shard  [4096 d, 1024 t]  resident in SBUF as fp32r (16.7 MB)
  - Wr packed  [32 ob, 128 d_in, 32 d_chunk, 128 o]  streamed per o-block
  - out yT     [4096 o, 1024 t]  written per o-block
Matmul dtype is float32r (full-rate on the PE at N>=256, ~1.5e-4 rel err).
Bias is fused into the PSUM->SBUF eviction via ScalarE activation.
"""
import sys

for _p in ("/opt/trn_rl_repo", "/root/.axon_site/_ro/trn_rl_repo"):
    if _p not in sys.path:
        sys.path.insert(0, _p)

import numpy as np

D = 4096          # feature dim (= rotation size)
TOKENS = 8192     # 4 * 2048
N_CORES = 8
T_CORE = TOKENS // N_CORES   # 1024 tokens per core
P = 128           # partitions
DC = D // P       # 32 contraction chunks
OB = D // P       # 32 output blocks
T_HALF = 512      # moving free-dim per matmul (fp32 max)

_compiled = None  # (nc, tmpdir) cache


def _matmul_hadU_np(x: np.ndarray) -> np.ndarray:
    """Normalized WHT along the last axis — exact port of the reference
    recursive-butterfly (K == 1 branch), in float64."""
    n = x.shape[-1]
    shape = x.shape
    v = x.reshape(-1, n, 1)
    while v.shape[1] > 1:
        b_, m, c = v.shape
        v = v.reshape(b_, m // 2, 2, c)
        a, b = v[:, :, 0, :], v[:, :, 1, :]
        v = np.concatenate([a + b, a - b], axis=-1)
    return v.reshape(shape) / np.sqrt(n)


def _build_nc():
    import concourse.tile as tile
    from concourse import bacc, mybir

    dt = mybir.dt
    nc = bacc.Bacc(None, target_bir_lowering=False)

    xt_d = nc.dram_tensor("xt", [D, T_CORE], dt.float32, kind="ExternalInput")
    w_d = nc.dram_tensor("w", [OB, P, DC, P], dt.float32, kind="ExternalInput")
    b_d = nc.dram_tensor("bias", [P, OB], dt.float32, kind="ExternalInput")
    y_d = nc.dram_tensor("yt", [D, T_CORE], dt.float32, kind="ExternalOutput")

    G0 = 3  # o-blocks processed c-outer in the startup group (PE saturates
            # while x tiles stream in; needs G0*2 PSUM banks + G0 W slots,
            # leaving one W slot free so the next block's W prefetches)
    DCH = DC // 2  # W tiles DMA'd in two halves for finer arrival granularity

    with tile.TileContext(nc) as tc:
        with (
            tc.tile_pool(name="xp", bufs=1) as xp,
            tc.tile_pool(name="wp", bufs=4, space="SBUF") as wp,
            tc.tile_pool(name="bp", bufs=1) as bp,
            tc.tile_pool(name="op", bufs=2) as op,
            tc.tile_pool(name="pp", bufs=4, space="PSUM") as pp,
        ):
            b_sb = bp.tile([P, OB], dt.float32)

            def load_w(ob):
                w_sb = wp.tile([P, DC, P], dt.float32r, tag="w", name=f"w_{ob}")
                nc.sync.dma_start(
                    w_sb[:, 0:DCH, :], w_d[ob, :, 0:DCH, :].bitcast(dt.float32r)
                )
                nc.sync.dma_start(
                    w_sb[:, DCH:DC, :], w_d[ob, :, DCH:DC, :].bitcast(dt.float32r)
                )
                return w_sb

            def load_x(c):
                t = xp.tile([P, T_CORE], dt.float32r, tag=f"x{c}", name=f"x_{c}")
                src = xt_d[c * P:(c + 1) * P, :].bitcast(dt.float32r)
                nc.sync.dma_start(t[:, 0:T_HALF], src[:, 0:T_HALF])
                nc.sync.dma_start(t[:, T_HALF:T_CORE], src[:, T_HALF:T_CORE])
                return t

            def mms(ps, w_sb, c, x_t):
                lhsT = w_sb[:, c, :]
                nc.tensor.matmul(
                    ps[:, 0:T_HALF], lhsT, x_t[:, 0:T_HALF],
                    start=(c == 0), stop=(c == DC - 1),
                )
                nc.tensor.matmul(
                    ps[:, T_HALF:T_CORE], lhsT, x_t[:, T_HALF:T_CORE],
                    start=(c == 0), stop=(c == DC - 1),
                )

            def evict(ob, ps):
                o_sb = op.tile([P, T_CORE], dt.float32, tag="o", name=f"o_{ob}")
                nc.scalar.activation(
                    o_sb[:], ps[:],
                    mybir.ActivationFunctionType.Identity,
                    bias=b_sb[:, ob:ob + 1],
                )
                nc.sync.dma_start(y_d[ob * P:(ob + 1) * P, :], o_sb[:])

            # issue order shapes DMA completion order: startup-group W first
            # (interleaved with the first x tiles), then the x stream
            w0_tiles = [load_w(ob) for ob in range(G0)]
            nc.sync.dma_start(b_sb[:], b_d[:])
            x_tiles = [load_x(c) for c in range(DC)]

            # startup group: c outer, o-blocks inner -> 2*G0 matmuls become
            # ready the moment x tile c lands
            ps0 = [
                pp.tile([P, T_CORE], dt.float32, tag="ps", name=f"ps0_{i}")
                for i in range(G0)
            ]
            for c in range(DC):
                for ob in range(G0):
                    mms(ps0[ob], w0_tiles[ob], c, x_tiles[c])
            for ob in range(G0):
                evict(ob, ps0[ob])

            # steady state: one o-block at a time, W prefetched one ahead
            for ob in range(G0, OB):
                w_sb = load_w(ob)
                ps = pp.tile([P, T_CORE], dt.float32, tag="ps", name=f"ps_{ob}")
                for c in range(DC):
                    mms(ps, w_sb, c, x_tiles[c])
                evict(ob, ps)

    nc.compile()
    return nc


def _get_nc():
    global _compiled
    if _compiled is None:
        _compiled = _build_nc()
    return _compiled


def _prep_inputs(x, W, b):
    x = np.asarray(x, dtype=np.float32)
    W = np.asarray(W, dtype=np.float32)
    b = np.asarray(b, dtype=np.float32)

    Wr = _matmul_hadU_np(W.astype(np.float64)).astype(np.float32)  # [o, d]
    # W_pack[ob, p, c, j] = Wr[ob*128 + j, c*128 + p]
    w_pack = np.ascontiguousarray(
        Wr.reshape(OB, P, DC, P).transpose(0, 3, 2, 1)
    )
    b_pack = np.ascontiguousarray(b.reshape(OB, P).T)  # [128, 32]

    xt = np.ascontiguousarray(
        x.reshape(N_CORES, T_CORE, D).transpose(0, 2, 1)
    )  # [8, 4096, 1024]

    in_maps = [
        {"xt": xt[c], "w": w_pack, "bias": b_pack} for c in range(N_CORES)
    ]
    return in_maps


def _assemble(results):
    # yt per core: [4096 o, 1024 t] -> y[t, o]
    parts = [np.ascontiguousarray(r["yt"].T) for r in results]
    y = np.concatenate(parts, axis=0)  # [8192, 4096]
    return y.reshape(4, 2048, D)


def _run(x, W, b, **spmd_kwargs):
    from concourse.bass_utils import run_bass_kernel_spmd

    nc = _get_nc()
    in_maps = _prep_inputs(x, W, b)
    res = run_bass_kernel_spmd(nc, in_maps, list(range(N_CORES)), **spmd_kwargs)
    return _assemble(res.results), res


def kernel(x, W, b):
    out, _ = _run(x, W, b)
    return out
